# revision 14
# baseline (speedup 1.0000x reference)
"""Masked multi-head attention on 8 NeuronCores (faithful torch raw-view semantics).

The reference reshapes (bs, sql, nh*edim) -> (bs, nh, sql, edim) as a RAW VIEW:
head h's length-1024 pseudo-sequence comes from x rows 128h..128h+127, each row
contributing 8 pseudo-positions s' = 8r + cb (cb = 256-col block of the
projection). Work splits into 32 independent (batch, head) pairs -> 4 per core.

Q^T/K^T/V^T are kept in NATURAL s' order (the PSUM->SBUF bias-copies scatter
columns via strided APs at zero extra cost), which makes the causal structure
block-aligned: score/PV/denominator matmuls only cover the unmasked column
suffix [p, 512) per key block (saves ~44% of attention PE columns), and the
mask shrinks to one shared [128,128] triangle added only on diagonal blocks.
V is transposed per 128-key natural block on the PE (bf16, via identity) so
P@V contracts keys on partitions; pt/V/oh/wot are bf16 (full PE rate at any
width, half DMA). Softmax denominator via ones-matmul accumulated per column
suffix; normalization multiplies by a reciprocal row and scatters straight
back to the PERMUTED oh layout the output projection wants. Q weights/bias
pre-scaled by 1/16. V projection runs fully in bf16 (it feeds bf16 V anyway),
Q/K projections and scores stay fp32r.
"""

import sys

sys.path.insert(0, "/opt/trn_rl_repo")

import ml_dtypes
import numpy as np

from concourse import bacc, mybir
from concourse.tile import TileContext
from concourse.bass_utils import run_bass_kernel_spmd

EDIM = 256
BS = 4
SQL = 1024
HPC = 4           # heads per core
NCORES = 8
FDT = mybir.dt.float32
RDT = mybir.dt.float32r    # matmul-feeding storage: full-rate PE, rounded fp32
BDT = mybir.dt.bfloat16
NEG = -1.0e30

_cache = {}


def _build():
    nc = bacc.Bacc(dynamic_dma_scratch_size=512)

    xtp = nc.declare_dram_parameter("xtp", [128, 1024], RDT, isOutput=False)
    xbp = nc.declare_dram_parameter("xbp", [128, 1024], BDT, isOutput=False)
    # V weights (bf16), d-packed: wva = both d's cb0 cols, wvb = the rest
    wva = nc.declare_dram_parameter("wva", [128, 512], BDT, isOutput=False)
    wvb = nc.declare_dram_parameter("wvb", [128, 3584], BDT, isOutput=False)
    # Q/16 and K weights (f32r): sections [Q/16, K], each 2048 wide
    wqk0 = nc.declare_dram_parameter("wqk0", [128, 4096], RDT, isOutput=False)
    wqk1 = nc.declare_dram_parameter("wqk1", [128, 4096], RDT, isOutput=False)
    bqkv = nc.declare_dram_parameter("bqkv", [128, 48], FDT, isOutput=False)
    wot = nc.declare_dram_parameter("wot", [128, 4096], BDT, isOutput=False)
    cst = nc.declare_dram_parameter("cst", [128, 384], BDT, isOutput=False)
    y = nc.declare_dram_parameter("y", [512, 256], FDT, isOutput=True)

    EXP = mybir.ActivationFunctionType.Exp

    with TileContext(nc) as tc:
        with (
            tc.tile_pool(name="const", bufs=1) as cpool,
            tc.tile_pool(name="wqkp", bufs=2) as wqkpool,
            tc.tile_pool(name="wvp", bufs=2) as wvpool,
            tc.tile_pool(name="qk", bufs=1) as qkpool,
            tc.tile_pool(name="ohp", bufs=2) as ohpool,
            tc.tile_pool(name="vkp", bufs=4) as vkpool,
            tc.tile_pool(name="ptp", bufs=6) as ptpool,
            tc.tile_pool(name="work", bufs=2) as wpool,
            tc.tile_pool(name="ps_a", bufs=4, space="PSUM") as ps_a,
            tc.tile_pool(name="ps_po", bufs=3, space="PSUM") as ps_po,
            tc.tile_pool(name="ps_se", bufs=1, space="PSUM") as ps_se,
        ):
            def load(pool, name, src, shape, dt=FDT, tag=None):
                t = pool.tile(shape, dt, tag=tag or name, name=name)
                nc.sync.dma_start(out=t[:, :], in_=src)
                return t

            def mm(out, lhsT, rhs, **kw):
                nc.tensor.matmul(out, lhsT, rhs, **kw)

            # DMA order = need order: x first, then V weights (bf16, consumed
            # first), Q/K weight chunks interleaved d0/d1, then the rest.
            # Startup issues alternate between the two HWDGE queues (SP, Act)
            # to halve the 650ns-per-issue serial chain.
            def tile_of(pool, name, shape, dt=FDT, tag=None):
                return pool.tile(shape, dt, tag=tag or name, name=name)

            xb_sb = tile_of(cpool, "xb", [128, 1024], dt=BDT)
            # wv tile layout: col = d*2048 + cb*256 + c*128 + d'
            wv_sb = tile_of(cpool, "wv", [128, 4096], dt=BDT)
            bqkv_sb = tile_of(cpool, "bqkv", [128, 48])
            xt_sb = tile_of(cpool, "xt", [128, 1024], dt=RDT)
            wv_r = wv_sb.rearrange("p (d o) -> p d o", d=2)
            nc.sync.dma_start(out=wv_r[:, :, 0:256], in_=wva[:, :].rearrange(
                "p (d o) -> p d o", d=2))
            nc.scalar.dma_start(out=xb_sb[:, :], in_=xbp[:, :])
            nc.sync.dma_start(out=wv_r[:, :, 256:2048], in_=wvb[:, :].rearrange(
                "p (d o) -> p d o", d=2))
            nc.scalar.dma_start(out=bqkv_sb[:, :], in_=bqkv[:, :])
            nc.sync.dma_start(out=xt_sb[:, :], in_=xtp[:, :])
            wqk_srcs = [wqk0, wqk1]
            wqk_sb = [wqkpool.tile([128, 4096], RDT, tag="wqk", name=f"wqk{d}")
                      for d in range(2)]
            for ch in range(4):
                for d in range(2):
                    eng = nc.sync if d == 0 else nc.scalar
                    eng.dma_start(
                        out=wqk_sb[d][:, ch * 1024:(ch + 1) * 1024],
                        in_=wqk_srcs[d][:, ch * 1024:(ch + 1) * 1024],
                    )
            cst_sb = load(cpool, "cst", cst[:, :], [128, 384], dt=BDT)
            idn_sb = cst_sb[:, 0:128]
            tri_sb = cst_sb[:, 128:256]
            onc_sb = cst_sb[:, 256:384]
            wot_sb = cpool.tile([128, 4096], BDT, tag="wot", name="wot")
            for ch in range(2):
                nc.sync.dma_start(
                    out=wot_sb[:, ch * 2048:(ch + 1) * 2048],
                    in_=wot[:, ch * 2048:(ch + 1) * 2048],
                )

            # PE warmup: 14 throwaway matmuls on never-written SBUF ramp the
            # PE to full pstate while the first weight DMAs are in flight.
            wup = wpool.tile([128, 512], RDT, tag="wup", name="wup", bufs=1)
            nc.gpsimd.memset(wup[:, :].bitcast(FDT), 0.0)
            wps = ps_a.tile([128, 512], FDT, tag="ps", name="warm")
            for _ in range(14):
                mm(wps[:, :], wup[:, 0:128], wup[:, :])

            # natural-order projections: col = h*1024 + 8r + cb
            qt = [qkpool.tile([128, 4096], RDT, tag=f"qt{c}", name=f"qt{c}")
                  for c in range(2)]
            kt = [qkpool.tile([128, 4096], RDT, tag=f"kt{c}", name=f"kt{c}")
                  for c in range(2)]
            vt = [qkpool.tile([128, 4096], BDT, tag=f"vt{c}", name=f"vt{c}")
                  for c in range(2)]

            # ---- P1: V^T, Q^T, K^T projections (V first: feeds transposes) ----
            p1_sections = [0, 1, 2]
            for s in p1_sections:       # 0 = V, 1 = Q, 2 = K
                dst = (vt, qt, kt)[s]
                for cb in range(8):
                    for c in range(2):
                        ti = s * 16 + cb * 2 + c
                        ps = ps_a.tile([128, 512], FDT, tag="ps", name="proj")
                        for d in range(2):
                            if s == 0:
                                w_ap = wv_sb[:, d * 2048 + cb * 256 + c * 128:
                                             d * 2048 + cb * 256 + c * 128
                                             + 128]
                                x_ap = xb_sb[:, d * 512:(d + 1) * 512]
                            else:
                                w_ap = wqk_sb[d][:, (s - 1) * 2048 + cb * 256
                                                 + c * 128:
                                                 (s - 1) * 2048 + cb * 256
                                                 + c * 128 + 128]
                                x_ap = xt_sb[:, d * 512:(d + 1) * 512]
                            mm(ps[:, :], w_ap, x_ap,
                               start=(d == 0), stop=(d == 1))
                        out_ap = dst[c].rearrange(
                            "p (h r e) -> p h r e", h=4, r=128)[:, :, :, cb]
                        in_ap = ps.rearrange("p (h r) -> p h r", h=4)[:, :, :]
                        if ti % 2 == 0:
                            nc.scalar.add(out_ap, in_ap,
                                          add=bqkv_sb[:, ti:ti + 1])
                        else:
                            nc.vector.tensor_scalar_add(
                                out=out_ap, in0=in_ap,
                                scalar1=bqkv_sb[:, ti:ti + 1],
                            )

            # normalized attention out, PERMUTED cols (= hl*1024 + cb*128 + r)
            oh = [ohpool.tile([128, 4096], BDT, tag="oh", name=f"oh{c}")
                  for c in range(2)]

            # V_k: natural 128-key blocks transposed to [key, d] per head
            def emit_transposes(hl):
                vkt = vkpool.tile([128, 2048], BDT, tag="vk", name=f"vk{hl}")
                for c in range(2):
                    tv = ps_a.tile([128, 512], FDT, tag="ps", name="tv")
                    tvb = tv[:, :].bitcast(BDT)
                    for k in range(8):
                        nc.tensor.transpose(
                            tvb[:, k * 128:(k + 1) * 128],
                            vt[c][:, hl * 1024 + k * 128:
                                  hl * 1024 + k * 128 + 128],
                            idn_sb,
                        )
                    nc.vector.tensor_copy(
                        out=vkt[:, c * 1024:(c + 1) * 1024], in_=tvb)
                return vkt

            vks = {0: emit_transposes(0)}

            for hl in range(HPC):
                vk = vks.pop(hl)
                # ---- attention, natural order, causal-suffix matmuls ----
                for qj in range(2):
                    if qj == 1 and hl + 1 < HPC:
                        # transpose next head's V while this head's big half
                        # computes (hides the vk SBUF-copy latency)
                        vks[hl + 1] = emit_transposes(hl + 1)
                    kmax = 4 * qj + 3
                    po = [ps_po.tile([128, 512], FDT, tag="po", name=f"po{c}")
                          for c in range(2)]
                    se = ps_se.tile([128, 512], FDT, tag="se", name="se")
                    for k in range(kmax + 1):
                        p_t = max(0, 128 * (k - 4 * qj))
                        p_m = min(p_t, 256)   # fp32r needs >=256 moving cols
                        sp = ps_a.tile([128, 512], FDT, tag="ps", name="score")
                        for c in range(2):
                            mm(
                                sp[:, p_m:512],
                                kt[c][:, hl * 1024 + k * 128:
                                      hl * 1024 + k * 128 + 128],
                                qt[c][:, hl * 1024 + 512 * qj + p_m:
                                      hl * 1024 + 512 * qj + 512],
                                start=(c == 0), stop=(c == 1),
                            )
                        if k >= 4 * qj:
                            nc.vector.tensor_add(
                                out=sp[:, p_t:p_t + 128],
                                in0=sp[:, p_t:p_t + 128],
                                in1=tri_sb,
                            )
                        pt = ptpool.tile([128, 512], BDT, tag="pt", name="pt")
                        nc.scalar.activation(pt[:, p_t:512], sp[:, p_t:512],
                                             EXP)
                        for c in range(2):
                            mm(
                                po[c][:, p_t:512],
                                vk[:, c * 1024 + k * 128:
                                   c * 1024 + k * 128 + 128],
                                pt[:, p_t:512],
                                start=(k == 0), stop=(k == kmax),
                            )
                        mm(
                            se[:, p_t:512], onc_sb, pt[:, p_t:512],
                            start=(k == 0), stop=(k == kmax),
                        )
                    rc = wpool.tile([128, 512], FDT, tag="rc", name="rc")
                    nc.vector.reciprocal(out=rc[:, :], in_=se[:, :])
                    for c in range(2):
                        for bh in range(2):
                            out_ap = oh[c].rearrange(
                                "p (h cb r) -> p h cb r", h=4, cb=8
                            )[:, hl, bh * 4:bh * 4 + 4, 64 * qj:64 * qj + 64]
                            nc.vector.tensor_mul(
                                out=out_ap,
                                in0=po[c][:, :].rearrange(
                                    "p (th cb) -> p cb th", cb=8
                                )[:, bh * 4:bh * 4 + 4, :],
                                in1=rc[:, :].rearrange(
                                    "p (th cb) -> p cb th", cb=8
                                )[:, bh * 4:bh * 4 + 4, :],
                            )

                # ---- output projection for this head (y rows are per-head) --
                yp = ps_se.tile([128, 512], FDT, tag="se", name="yp")
                # c-major order: the first 8 accumulations only need oh[0],
                # so P4 starts one normalize-mul earlier
                for jj in range(16):
                    c, cb = divmod(jj, 8)
                    j = 2 * cb + c
                    mm(
                        yp[:, 0:256],
                        oh[c][:, hl * 1024 + cb * 128:
                              hl * 1024 + cb * 128 + 128],
                        wot_sb[:, j * 256:(j + 1) * 256],
                        start=(jj == 0), stop=(jj == 15),
                    )
                ys = wpool.tile([128, 256], FDT, tag="ys", name="ys")
                nc.scalar.copy(out=ys[:, :], in_=yp[:, 0:256])
                nc.scalar.dma_start(
                    out=y[hl * 128:(hl + 1) * 128, :], in_=ys[:, :]
                )
    nc.finalize()
    return nc


def _prep_inputs(x, w_attn, b_attn, w_out):
    # shared diagonal-block causal mask in natural order: masked iff key > query
    r = np.arange(128)
    tri_arr = np.where(r[:, None] <= r[None, :], 0.0, NEG).astype(
        ml_dtypes.bfloat16)

    wv = np.ascontiguousarray(w_attn[4096:6144].T).astype(
        ml_dtypes.bfloat16)      # (256, 2048)
    wva_arr = np.ascontiguousarray(
        np.concatenate([wv[:128, 0:256], wv[128:, 0:256]], axis=1))
    wvb_arr = np.ascontiguousarray(
        np.concatenate([wv[:128, 256:2048], wv[128:, 256:2048]], axis=1))
    cst_arr = np.ascontiguousarray(np.concatenate(
        [np.eye(128, dtype=ml_dtypes.bfloat16),
         tri_arr,
         np.ones((128, 128), ml_dtypes.bfloat16)], axis=1))
    wqk = np.ascontiguousarray(
        np.concatenate([w_attn[0:2048] / 16.0, w_attn[2048:4096]]).T
    )  # (256, 4096), sections [Q/16, K]
    bqkv_arr = np.ascontiguousarray(
        np.concatenate([b_attn[4096:6144], b_attn[0:2048] / 16.0,
                        b_attn[2048:4096]]).reshape(48, 128).T
    ).astype(np.float32)  # (128, 48), tile order [V, Q, K]
    wot_arr = np.ascontiguousarray(
        w_out.T.reshape(16, 128, 256).transpose(1, 0, 2).reshape(128, 4096)
    ).astype(ml_dtypes.bfloat16)

    in_maps = []
    for c in range(NCORES):
        b, g = divmod(c, 2)
        xt = np.ascontiguousarray(x[b, 512 * g:512 * (g + 1)].T)  # (256, 512)
        xp = np.ascontiguousarray(
            np.concatenate([xt[:128], xt[128:]], axis=1))
        in_maps.append({
            "xtp": xp,
            "xbp": xp.astype(ml_dtypes.bfloat16),
            "wva": wva_arr,
            "wvb": wvb_arr,
            "wqk0": np.ascontiguousarray(wqk[:128]),
            "wqk1": np.ascontiguousarray(wqk[128:]),
            "bqkv": bqkv_arr,
            "wot": wot_arr,
            "cst": cst_arr,
        })
    return in_maps


def kernel(x, w_attn, b_attn, w_out, b_out):
    x = np.asarray(x, dtype=np.float32)
    w_attn = np.asarray(w_attn, dtype=np.float32)
    b_attn = np.asarray(b_attn, dtype=np.float32)
    w_out = np.asarray(w_out, dtype=np.float32)
    b_out = np.asarray(b_out, dtype=np.float32)

    if "nc" not in _cache:
        _cache["nc"] = _build()
    nc = _cache["nc"]

    in_maps = _prep_inputs(x, w_attn, b_attn, w_out)
    res = run_bass_kernel_spmd(nc, in_maps, list(range(NCORES))).results

    out = np.empty((BS, SQL, EDIM), dtype=np.float32)
    for c in range(NCORES):
        b, g = divmod(c, 2)
        out[b, 512 * g:512 * (g + 1)] = res[c]["y"]
    out += b_out
    return out
